# revision 13
# baseline (speedup 1.0000x reference)
"""Trainium2 Bass kernel for GAT attention mechanism.

Reference computation (N=1024, F=F'=128):
    Vp    = V @ W.T + b                       # [N, F']
    score = s_i[:,None] + s_j[None,:] + einsum('ijf,f->ij', time_enc, a_t)
    score = leaky_relu(score, 0.01)
    att   = softmax(score, axis=-1)
    H     = att[:,:,None] * Vp[None,:,:] * E[:,:,None]   # [N, N, F']

Sharding: row-blocks of i across 8 cores (128 rows each). Each core gets its
slice of time_enc/E plus replicated V/params; softmax over j stays local.
"""

import sys

sys.path.insert(0, "/opt/trn_rl_repo")

import numpy as np

import concourse.bass as bass
import concourse.tile as tile
from concourse import bacc, mybir
from concourse.masks import make_identity

F32 = mybir.dt.float32

N = 1024          # nodes (j dimension, full)
F = 128           # feature dim (= F_OUT = F_IN)
M = 8             # cores
NL = N // M       # local i rows per core (128)
JC = 64           # j columns per time_enc chunk
NCH = N // JC     # chunks
IB = 2            # i rows batched per H store DMA
NEG_SLOPE = 0.01


def build_kernel(n=N, nl=NL, jc=JC):
    nch = n // jc
    nt = n // 128  # number of 128-wide j tiles

    nc = bacc.Bacc()
    te_d = nc.dram_tensor("te", [nl, n, F], F32, kind="ExternalInput")
    E_d = nc.dram_tensor("E", [nl, n], F32, kind="ExternalInput")
    V_d = nc.dram_tensor("V", [n, F], F32, kind="ExternalInput")
    Vloc_d = nc.dram_tensor("Vloc", [nl, F], F32, kind="ExternalInput")
    W_d = nc.dram_tensor("W", [F, F], F32, kind="ExternalInput")
    b_d = nc.dram_tensor("b", [F, 1], F32, kind="ExternalInput")
    a_d = nc.dram_tensor("a", [3 * F, 1], F32, kind="ExternalInput")
    H_d = nc.dram_tensor("H", [nl, n, F], F32, kind="ExternalOutput")

    mult = mybir.AluOpType.mult
    add = mybir.AluOpType.add
    amax = mybir.AluOpType.max

    with tile.TileContext(nc) as tc:
        with (
            tc.tile_pool(name="const", bufs=1) as cp,
            tc.tile_pool(name="psum", bufs=2, space="PSUM") as pp,
            tc.tile_pool(name="te", bufs=2) as tep,
            tc.tile_pool(name="scratch", bufs=2) as scr,
            tc.tile_pool(name="hout", bufs=4) as hp,
        ):
            # ---- constants -------------------------------------------------
            ident = cp.tile([128, 128], F32)
            make_identity(nc, ident)
            ones_row = cp.tile([1, 128], F32)
            nc.vector.memset(ones_row, 1.0)

            b_col = cp.tile([F, 1], F32)
            nc.sync.dma_start(b_col, b_d[:, :])
            a_i_col = cp.tile([F, 1], F32)
            nc.sync.dma_start(a_i_col, a_d[0:F, :])
            a_j_col = cp.tile([F, 1], F32)
            nc.sync.dma_start(a_j_col, a_d[F : 2 * F, :])
            # a_t replicated to all partitions: [128, F] via broadcast DMA
            a_t_rep = cp.tile([128, F], F32)
            a_sl = a_d[2 * F : 3 * F, 0:1]
            a_t_src = bass.AP(
                tensor=a_sl.tensor,
                offset=a_sl.offset,
                ap=[[0, 128], [1, F]],
            )
            nc.gpsimd.dma_start(out=a_t_rep, in_=a_t_src)

            W_sb = cp.tile([F, F], F32)
            nc.sync.dma_start(W_sb, W_d[:, :])
            Wt_sb = cp.tile([F, F], F32)  # [f, f'] = W.T
            ps = pp.tile([128, 128], F32, tag="tp")
            nc.tensor.transpose(ps, W_sb, ident)
            nc.scalar.copy(Wt_sb, ps)

            # V in [n, F]; tiles of 128 rows: V_sb[p, t, f] = V[t*128+p, f]
            V_sb = cp.tile([128, nt, F], F32)
            nc.sync.dma_start(V_sb, V_d[:, :].rearrange("(t p) f -> p t f", p=128))
            # V^T: VT[f, t, p] = V[t*128+p, f]
            VT_sb = cp.tile([F, nt, 128], F32)
            for t in range(nt):
                ps = pp.tile([128, 128], F32, tag="tp")
                nc.tensor.transpose(ps, V_sb[:, t, :], ident)
                nc.scalar.copy(VT_sb[:, t, :], ps)

            # Vp^T[f', nn] = sum_f Wt[f, f'] * VT[f, nn]  (+ bias b[f'])
            VpT_sb = cp.tile([F, nt, 128], F32)
            for h in range(0, nt, 4):
                hw = min(4, nt - h)
                psw = pp.tile([128, 512], F32, tag="mm")
                nc.tensor.matmul(
                    psw[:, : hw * 128],
                    Wt_sb,
                    VT_sb[:, h : h + hw, :].rearrange("p a b -> p (a b)"),
                )
                nc.scalar.activation(
                    VpT_sb[:, h : h + hw, :].rearrange("p a b -> p (a b)"),
                    psw[:, : hw * 128],
                    mybir.ActivationFunctionType.Identity,
                    bias=b_col, scale=1.0,
                )

            # Vp[p, t, f'] = Vp[t*128+p, f']
            Vp_sb = cp.tile([128, nt, F], F32)
            for t in range(nt):
                ps = pp.tile([128, 128], F32, tag="tp")
                nc.tensor.transpose(ps, VpT_sb[:, t, :], ident)
                nc.scalar.copy(Vp_sb[:, t, :], ps)

            # local rows: VpT_loc[f', i] then s_i[i] = sum Vp_loc[i,:]*a_i
            Vloc_sb = cp.tile([nl, F], F32)
            nc.sync.dma_start(Vloc_sb, Vloc_d[:, :])
            VlocT_sb = cp.tile([F, nl], F32)
            ps = pp.tile([128, 128], F32, tag="tp")
            nc.tensor.transpose(ps, Vloc_sb, ident)
            nc.scalar.copy(VlocT_sb, ps)
            VpTloc_sb = cp.tile([F, nl], F32)
            ps = pp.tile([128, 128], F32, tag="tp")
            nc.tensor.matmul(ps, Wt_sb, VlocT_sb)
            nc.scalar.activation(
                VpTloc_sb, ps, mybir.ActivationFunctionType.Identity,
                bias=b_col, scale=1.0,
            )
            s_i_col = cp.tile([nl, 1], F32)
            ps_si = pp.tile([128, 1], F32, tag="mm")
            nc.tensor.matmul(ps_si[:nl, :], VpTloc_sb, a_i_col)
            nc.vector.tensor_copy(s_i_col, ps_si[:nl, :])

            # s_j row then broadcast across partitions
            mw = min(512, n)
            sj_row = cp.tile([1, n], F32)
            for h in range(0, n, mw):
                ps_sj = pp.tile([1, mw], F32, tag="mm")
                nc.tensor.matmul(
                    ps_sj, a_j_col,
                    VpT_sb[:, :, :].rearrange("f t p -> f (t p)")[:, h : h + mw],
                )
                nc.scalar.copy(sj_row[:, h : h + mw], ps_sj)
            sjb_sb = cp.tile([128, n], F32)
            for h in range(0, n, mw):
                psw = pp.tile([128, mw], F32, tag="mm")
                nc.tensor.matmul(psw, ones_row, sj_row[:, h : h + mw])
                nc.scalar.copy(sjb_sb[:, h : h + mw], psw)

            E_sb = cp.tile([nl, n], F32)
            nc.sync.dma_start(E_sb, E_d[:, :])

            # ---- phase A: scores -------------------------------------------
            te_s = cp.tile([nl, n], F32)
            for c in range(nch):
                te_t = tep.tile([nl, jc, F], F32, tag="te")
                nc.sync.dma_start(te_t, te_d[:, c * jc : (c + 1) * jc, :])
                for jj in range(jc):
                    j = c * jc + jj
                    prod = scr.tile([nl, F], F32, tag="prod")
                    nc.vector.scalar_tensor_tensor(
                        out=prod,
                        in0=te_t[:, jj, :],
                        scalar=0.0,
                        in1=a_t_rep[:nl, :],
                        op0=add,
                        op1=mult,
                        accum_out=te_s[:, j : j + 1],
                    )

            # ---- softmax tail ----------------------------------------------
            # score = te_s + s_j (bcast) + s_i; score2 = leaky_relu(score)
            score_pre = cp.tile([nl, n], F32)
            nc.vector.tensor_add(score_pre, te_s, sjb_sb[:nl, :])
            score2 = cp.tile([nl, n], F32)
            score_lo = cp.tile([nl, n], F32)
            nc.vector.tensor_scalar(
                out=score_lo, in0=score_pre, scalar1=s_i_col, scalar2=NEG_SLOPE,
                op0=add, op1=mult,
            )
            nc.vector.scalar_tensor_tensor(
                out=score2, in0=score_pre, scalar=s_i_col, in1=score_lo,
                op0=add, op1=amax,
            )
            neg_max = cp.tile([nl, 1], F32)
            nc.vector.tensor_reduce(
                neg_max, score2, axis=mybir.AxisListType.X, op=amax, negate=True,
            )
            exps = cp.tile([nl, n], F32)
            row_sum = cp.tile([nl, 1], F32)
            nc.scalar.activation(
                exps, score2, mybir.ActivationFunctionType.Exp,
                bias=neg_max, scale=1.0, accum_out=row_sum,
            )
            rinv = cp.tile([nl, 1], F32)
            nc.vector.reciprocal(rinv, row_sum)
            w_sb = cp.tile([nl, n], F32)
            nc.vector.tensor_mul(w_sb, exps, E_sb)
            nc.vector.tensor_scalar_mul(w_sb, w_sb, rinv)

            # wT[p, t, i] = w[i, t*128+p]
            wT_sb = cp.tile([128, nt, nl], F32)
            for t in range(nt):
                ps = pp.tile([128, 128], F32, tag="tp")
                nc.tensor.transpose(ps[:, :nl], w_sb[:, t * 128 : (t + 1) * 128], ident)
                nc.scalar.copy(wT_sb[:, t, :], ps[:, :nl])

            # ---- phase B: H ------------------------------------------------
            for g in range(nl // IB):
                H_t = hp.tile([128, IB, nt, F], F32, tag="H")
                for ii in range(IB):
                    i = g * IB + ii
                    for t in range(nt):
                        nc.vector.tensor_scalar_mul(
                            H_t[:, ii, t, :], Vp_sb[:, t, :],
                            wT_sb[:, t, i : i + 1],
                        )
                nc.sync.dma_start(
                    H_d[g * IB : (g + 1) * IB].rearrange(
                        "i (t p) f -> p i t f", p=128
                    ),
                    H_t,
                )

    nc.compile()
    return nc


_NC_CACHE = {}


def _get_nc():
    if "nc" not in _NC_CACHE:
        _NC_CACHE["nc"] = build_kernel()
    return _NC_CACHE["nc"]


def make_in_maps(V, E, time_enc, W_weight, W_bias, a):
    V = np.asarray(V, dtype=np.float32)
    E = np.asarray(E, dtype=np.float32)
    time_enc = np.asarray(time_enc, dtype=np.float32)
    W_weight = np.asarray(W_weight, dtype=np.float32)
    W_bias = np.asarray(W_bias, dtype=np.float32).reshape(F, 1)
    a = np.asarray(a, dtype=np.float32)
    in_maps = []
    for c in range(M):
        sl = slice(c * NL, (c + 1) * NL)
        in_maps.append(
            {
                "te": np.ascontiguousarray(time_enc[sl]),
                "E": np.ascontiguousarray(E[sl]),
                "V": V,
                "Vloc": np.ascontiguousarray(V[sl]),
                "W": W_weight,
                "b": W_bias,
                "a": a,
            }
        )
    return in_maps


def kernel(V, E, time_enc, W_weight, W_bias, a):
    from concourse.bass_utils import run_bass_kernel_spmd

    nc = _get_nc()
    in_maps = make_in_maps(V, E, time_enc, W_weight, W_bias, a)
    res = run_bass_kernel_spmd(nc, in_maps, core_ids=list(range(M)))
    return np.concatenate([res.results[c]["H"] for c in range(M)], axis=0)


# revision 16
# speedup vs baseline: 5.1014x; 5.1014x over previous
"""Trainium2 Bass kernel for GAT attention mechanism.

Reference computation (N=1024, F=F'=128):
    Vp    = V @ W.T + b                       # [N, F']
    score = s_i[:,None] + s_j[None,:] + einsum('ijf,f->ij', time_enc, a_t)
    score = leaky_relu(score, 0.01)
    att   = softmax(score, axis=-1)
    H     = att[:,:,None] * Vp[None,:,:] * E[:,:,None]   # [N, N, F']

Sharding: row-blocks of i across 8 cores (128 rows each). Each core gets its
slice of time_enc/E plus replicated V/params; softmax over j stays local.
"""

import sys

sys.path.insert(0, "/opt/trn_rl_repo")

import numpy as np

import concourse.bass as bass
import concourse.tile as tile
from concourse import bacc, mybir
from concourse.masks import make_identity

F32 = mybir.dt.float32

N = 1024          # nodes (j dimension, full)
F = 128           # feature dim (= F_OUT = F_IN)
M = 8             # cores
NL = N // M       # local i rows per core (128)
JC = 64           # j columns per time_enc chunk
NCH = N // JC     # chunks
IB = 2            # i rows batched per H store DMA
NEG_SLOPE = 0.01


def build_kernel(n=N, nl=NL, jc=JC, reps=1):
    nch = n // jc
    nt = n // 128  # number of 128-wide j tiles

    nc = bacc.Bacc()
    te_d = nc.dram_tensor("te", [nl, n, F], F32, kind="ExternalInput")
    E_d = nc.dram_tensor("E", [nl, n], F32, kind="ExternalInput")
    V_d = nc.dram_tensor("V", [n, F], F32, kind="ExternalInput")
    Vloc_d = nc.dram_tensor("Vloc", [nl, F], F32, kind="ExternalInput")
    W_d = nc.dram_tensor("W", [F, F], F32, kind="ExternalInput")
    b_d = nc.dram_tensor("b", [F, 1], F32, kind="ExternalInput")
    a_d = nc.dram_tensor("a", [3 * F, 1], F32, kind="ExternalInput")
    H_d = nc.dram_tensor("H", [nl, n, F], F32, kind="ExternalOutput")

    mult = mybir.AluOpType.mult
    add = mybir.AluOpType.add
    amax = mybir.AluOpType.max

    with tile.TileContext(nc) as tc:
        from contextlib import nullcontext
        with (
            tc.tile_pool(name="const", bufs=1) as cp,
            tc.tile_pool(name="psum", bufs=2, space="PSUM") as pp,
            tc.tile_pool(name="te", bufs=2) as tep,
            tc.tile_pool(name="scratch", bufs=2) as scr,
            tc.tile_pool(name="hout", bufs=4) as hp,
            tc.tile_pool(name="wt", bufs=2) as wtp,
            tc.For_i(0, reps, 1) if reps > 1 else nullcontext(),
        ):
            # ---- early te prefetch (before const chain; DMA FIFO order) ----
            te_tiles = {}
            for c in range(min(2, nch)):
                te_t = tep.tile([nl, jc, F], F32, tag="te")
                nc.sync.dma_start(te_t, te_d[:, c * jc : (c + 1) * jc, :])
                te_tiles[c] = te_t

            # ---- constants -------------------------------------------------
            ident = cp.tile([128, 128], F32)
            make_identity(nc, ident)
            ones_row = cp.tile([1, 128], F32)
            nc.vector.memset(ones_row, 1.0)

            b_col = cp.tile([F, 1], F32)
            nc.sync.dma_start(b_col, b_d[:, :])
            a_i_col = cp.tile([F, 1], F32)
            nc.sync.dma_start(a_i_col, a_d[0:F, :])
            a_j_col = cp.tile([F, 1], F32)
            nc.sync.dma_start(a_j_col, a_d[F : 2 * F, :])
            # a_t replicated to all partitions: [128, F] via broadcast DMA
            a_t_rep = cp.tile([128, F], F32)
            a_sl = a_d[2 * F : 3 * F, 0:1]
            a_t_src = bass.AP(
                tensor=a_sl.tensor,
                offset=a_sl.offset,
                ap=[[0, 128], [1, F]],
            )
            nc.gpsimd.dma_start(out=a_t_rep, in_=a_t_src)

            W_sb = cp.tile([F, F], F32)
            nc.sync.dma_start(W_sb, W_d[:, :])
            Wt_sb = cp.tile([F, F], F32)  # [f, f'] = W.T
            ps = pp.tile([128, 128], F32, tag="tp")
            nc.tensor.transpose(ps, W_sb, ident)
            nc.scalar.copy(Wt_sb, ps)

            # V in [n, F]; tiles of 128 rows: V_sb[p, t, f] = V[t*128+p, f]
            V_sb = cp.tile([128, nt, F], F32)
            nc.sync.dma_start(V_sb, V_d[:, :].rearrange("(t p) f -> p t f", p=128))
            # V^T: VT[f, t, p] = V[t*128+p, f]
            VT_sb = cp.tile([F, nt, 128], F32)
            for t in range(nt):
                ps = pp.tile([128, 128], F32, tag="tp")
                nc.tensor.transpose(ps, V_sb[:, t, :], ident)
                nc.scalar.copy(VT_sb[:, t, :], ps)

            # Vp^T[f', nn] = sum_f Wt[f, f'] * VT[f, nn]  (+ bias b[f'])
            VpT_sb = cp.tile([F, nt, 128], F32)
            for h in range(0, nt, 4):
                hw = min(4, nt - h)
                psw = pp.tile([128, 512], F32, tag="mm")
                nc.tensor.matmul(
                    psw[:, : hw * 128],
                    Wt_sb,
                    VT_sb[:, h : h + hw, :].rearrange("p a b -> p (a b)"),
                )
                nc.scalar.activation(
                    VpT_sb[:, h : h + hw, :].rearrange("p a b -> p (a b)"),
                    psw[:, : hw * 128],
                    mybir.ActivationFunctionType.Identity,
                    bias=b_col, scale=1.0,
                )

            # Vp[p, t, f'] = Vp[t*128+p, f']
            Vp_sb = cp.tile([128, nt, F], F32)
            for t in range(nt):
                ps = pp.tile([128, 128], F32, tag="tp")
                nc.tensor.transpose(ps, VpT_sb[:, t, :], ident)
                nc.scalar.copy(Vp_sb[:, t, :], ps)

            # local rows: VpT_loc[f', i] then s_i[i] = sum Vp_loc[i,:]*a_i
            Vloc_sb = cp.tile([nl, F], F32)
            nc.sync.dma_start(Vloc_sb, Vloc_d[:, :])
            VlocT_sb = cp.tile([F, nl], F32)
            ps = pp.tile([128, 128], F32, tag="tp")
            nc.tensor.transpose(ps, Vloc_sb, ident)
            nc.scalar.copy(VlocT_sb, ps)
            VpTloc_sb = cp.tile([F, nl], F32)
            ps = pp.tile([128, 128], F32, tag="tp")
            nc.tensor.matmul(ps, Wt_sb, VlocT_sb)
            nc.scalar.activation(
                VpTloc_sb, ps, mybir.ActivationFunctionType.Identity,
                bias=b_col, scale=1.0,
            )
            s_i_col = cp.tile([nl, 1], F32)
            ps_si = pp.tile([128, 1], F32, tag="mm")
            nc.tensor.matmul(ps_si[:nl, :], VpTloc_sb, a_i_col)
            nc.vector.tensor_copy(s_i_col, ps_si[:nl, :])

            # s_j row then broadcast across partitions
            mw = min(512, n)
            sj_row = cp.tile([1, n], F32)
            for h in range(0, n, mw):
                ps_sj = pp.tile([1, mw], F32, tag="mm")
                nc.tensor.matmul(
                    ps_sj, a_j_col,
                    VpT_sb[:, :, :].rearrange("f t p -> f (t p)")[:, h : h + mw],
                )
                nc.scalar.copy(sj_row[:, h : h + mw], ps_sj)
            sjb_sb = cp.tile([128, n], F32)
            for h in range(0, n, mw):
                psw = pp.tile([128, mw], F32, tag="mm")
                nc.tensor.matmul(psw, ones_row, sj_row[:, h : h + mw])
                nc.scalar.copy(sjb_sb[:, h : h + mw], psw)

            E_sb = cp.tile([nl, n], F32)
            nc.sync.dma_start(E_sb, E_d[:, :])

            # ---- phase A: scores -------------------------------------------
            te_s = cp.tile([nl, n], F32)
            for c in range(nch):
                if c in te_tiles:
                    te_t = te_tiles[c]
                else:
                    te_t = tep.tile([nl, jc, F], F32, tag="te")
                    nc.sync.dma_start(te_t, te_d[:, c * jc : (c + 1) * jc, :])
                for jj in range(jc):
                    j = c * jc + jj
                    prod = scr.tile([nl, F], F32, tag="prod")
                    nc.vector.scalar_tensor_tensor(
                        out=prod,
                        in0=te_t[:, jj, :],
                        scalar=0.0,
                        in1=a_t_rep[:nl, :],
                        op0=add,
                        op1=mult,
                        accum_out=te_s[:, j : j + 1],
                    )

            # ---- softmax tail ----------------------------------------------
            # score = te_s + s_i + s_j (bcast); score2 = leaky_relu(score)
            score_pre = cp.tile([nl, n], F32)
            nc.vector.scalar_tensor_tensor(
                out=score_pre, in0=te_s, scalar=s_i_col, in1=sjb_sb[:nl, :],
                op0=add, op1=add,
            )
            score2 = cp.tile([nl, n], F32)
            score_lo = cp.tile([nl, n], F32)
            nc.vector.tensor_scalar_mul(
                out=score_lo, in0=score_pre, scalar1=NEG_SLOPE,
            )
            nc.vector.tensor_tensor(
                out=score2, in0=score_pre, in1=score_lo, op=amax,
            )
            neg_max = cp.tile([nl, 1], F32)
            nc.vector.tensor_reduce(
                neg_max, score2, axis=mybir.AxisListType.X, op=amax, negate=True,
            )
            exps = cp.tile([nl, n], F32)
            row_sum = cp.tile([nl, 1], F32)
            nc.scalar.activation(
                exps, score2, mybir.ActivationFunctionType.Exp,
                bias=neg_max, scale=1.0, accum_out=row_sum,
            )
            rinv = cp.tile([nl, 1], F32)
            nc.vector.reciprocal(rinv, row_sum)
            w_sb = cp.tile([nl, n], F32)
            nc.vector.tensor_mul(w_sb, exps, E_sb)
            nc.vector.tensor_scalar_mul(w_sb, w_sb, rinv)

            # ---- phase B: H, ordered by j-tile so stores start early -------
            # wT_t[p, i] = w[i, t*128+p]; H[i, j, f] = w[i, j] * Vp[j, f]
            igb = 16  # i rows per store tile (1 MB per DMA)
            for t in range(nt):
                ps = pp.tile([128, 128], F32, tag="tp")
                nc.tensor.transpose(ps[:, :nl], w_sb[:, t * 128 : (t + 1) * 128], ident)
                wT_t = wtp.tile([128, nl], F32, tag="wt")
                nc.scalar.copy(wT_t, ps[:, :nl])
                for ig in range(nl // igb):
                    H_t = hp.tile([128, igb, F], F32, tag="H")
                    for ii in range(igb):
                        i = ig * igb + ii
                        nc.vector.tensor_scalar_mul(
                            H_t[:, ii, :], Vp_sb[:, t, :], wT_t[:, i : i + 1],
                        )
                    nc.sync.dma_start(
                        H_d[
                            ig * igb : (ig + 1) * igb,
                            t * 128 : (t + 1) * 128,
                            :,
                        ].rearrange("i p f -> p i f"),
                        H_t,
                    )

    nc.compile()
    return nc


_NC_CACHE = {}


def _get_nc():
    if "nc" not in _NC_CACHE:
        _NC_CACHE["nc"] = build_kernel()
    return _NC_CACHE["nc"]


def make_in_maps(V, E, time_enc, W_weight, W_bias, a):
    V = np.asarray(V, dtype=np.float32)
    E = np.asarray(E, dtype=np.float32)
    time_enc = np.asarray(time_enc, dtype=np.float32)
    W_weight = np.asarray(W_weight, dtype=np.float32)
    W_bias = np.asarray(W_bias, dtype=np.float32).reshape(F, 1)
    a = np.asarray(a, dtype=np.float32)
    in_maps = []
    for c in range(M):
        sl = slice(c * NL, (c + 1) * NL)
        in_maps.append(
            {
                "te": np.ascontiguousarray(time_enc[sl]),
                "E": np.ascontiguousarray(E[sl]),
                "V": V,
                "Vloc": np.ascontiguousarray(V[sl]),
                "W": W_weight,
                "b": W_bias,
                "a": a,
            }
        )
    return in_maps


def kernel(V, E, time_enc, W_weight, W_bias, a):
    from concourse.bass_utils import run_bass_kernel_spmd

    nc = _get_nc()
    in_maps = make_in_maps(V, E, time_enc, W_weight, W_bias, a)
    res = run_bass_kernel_spmd(nc, in_maps, core_ids=list(range(M)))
    return np.concatenate([res.results[c]["H"] for c in range(M)], axis=0)
